# revision 6
# baseline (speedup 1.0000x reference)
"""MoE layer (top-2 of 8, SwiGLU) on 8 TRN2 cores — dff-split load balancing.

Each expert's FFN is split into two d_ff halves (2048 each); the 16
(expert, half) shards are sorted by padded token count and dealt so every
core gets one large-slot shard (C_A tokens) and one small-slot shard
(C_B tokens).  Pairing big experts' shards with small ones cuts the
critical core from 2176 full-token equivalents to (C_A + C_B)/2 = 2112.
SwiGLU is elementwise in d_ff, so each shard's partial y is exact; the
host sums the two halves per expert before the router-weighted combine.
"""

import numpy as np
import ml_dtypes

import concourse.bass as bass
import concourse.mybir as mybir
import concourse.tile as tile
from concourse.bass_utils import run_bass_kernel_spmd

_ws_counter = [0]


def _split_multi_waits(nc: bass.Bass) -> int:
    n_split = 0
    for f in nc.m.functions:
        for bb in f.blocks:
            new_insts = []
            for inst in bb.instructions:
                si = inst.sync_info
                if si is not None and si.on_wait and len(si.on_wait) > 1:
                    waits = list(si.on_wait)
                    for w in waits[:-1]:
                        _ws_counter[0] += 1
                        n_split += 1
                        new_insts.append(
                            mybir.InstNoOp(
                                name=f"waitsplit-{_ws_counter[0]}",
                                opcode="NoOp",
                                engine=inst.engine,
                                sync_info=mybir.SyncInfo(on_wait=[w], on_update=[]),
                                bass_nofuse=True,
                                text_hint="waitsplit",
                            )
                        )
                    si.on_wait = [waits[-1]]
                new_insts.append(inst)
            bb.instructions[:] = new_insts
    return n_split


D = 1024
DFF = 2048          # per-shard d_ff half
N_EXPERTS = 8
TOP_K = 2
N_CORES = 8
TB = 512
KD = D // 128       # 8 contraction tiles over d
NF = DFF // 128     # 16 tiles over the d_ff half

BF16 = mybir.dt.bfloat16
F32 = mybir.dt.float32
NP_BF16 = ml_dtypes.bfloat16

_NC_CACHE: dict[tuple, bass.Bass] = {}


def _blocks_for(C: int) -> list[int]:
    blocks = [TB] * (C // TB)
    r = C % TB
    if r == 128:
        if blocks:
            blocks = blocks[:-1] + [384, 256]
        else:
            blocks = [128]
    elif r:
        blocks.append(r)
    blocks.sort()
    return blocks


def _build_kernel(CA: int, CB: int, repeat: int = 1) -> bass.Bass:
    assert CA % 128 == 0 and CB % 128 == 0
    nc = bass.Bass()
    segs = []
    for s, C in (("a", CA), ("b", CB)):
        segs.append(
            dict(
                C=C,
                xt=nc.dram_tensor(f"xt_{s}", [128, KD, C], BF16, kind="ExternalInput"),
                w1t=nc.dram_tensor(f"w1t_{s}", [128, KD, DFF], BF16, kind="ExternalInput"),
                wgt=nc.dram_tensor(f"wgt_{s}", [128, KD, DFF], BF16, kind="ExternalInput"),
                w2t=nc.dram_tensor(f"w2t_{s}", [128, NF, D], BF16, kind="ExternalInput"),
                y=nc.dram_tensor(f"y_{s}", [C, D], F32, kind="ExternalOutput"),
            )
        )

    silu = mybir.ActivationFunctionType.Silu

    with tile.TileContext(nc) as tc:
        with (
            tc.tile_pool(name="wres", bufs=1) as wres,
            tc.tile_pool(name="wg", bufs=4) as wgpool,
            tc.tile_pool(name="xt", bufs=3) as xtpool,
            tc.tile_pool(name="hg", bufs=3) as hgpool,
            tc.tile_pool(name="h", bufs=24) as hpool,
            tc.tile_pool(name="w2", bufs=12) as w2pool,
            tc.tile_pool(name="yo", bufs=4) as ypool,
            tc.tile_pool(name="ps1", bufs=1, space="PSUM") as psum1,
            tc.tile_pool(name="ps2", bufs=4, space="PSUM") as psum2,
        ):
            # Resident w1 per segment, in 512-col parts (NF//4 = 4 per seg).
            for si, seg in enumerate(segs):
                seg["w1_parts"] = [
                    wres.tile([128, KD, 512], BF16, tag=f"w1p{si}_{i}",
                              name=f"w1p{si}_{i}")
                    for i in range(NF // 4)
                ]

            if repeat > 1:
                for seg in segs:
                    for i in range(NF // 4):
                        nc.sync.dma_start(
                            seg["w1_parts"][i][:],
                            seg["w1t"][:, :, i * 512:(i + 1) * 512],
                        )

            def _seg_body(seg, first_seg):
              tok0 = 0
              for b, tb in enumerate(_blocks_for(seg["C"])):
                xt_sb = xtpool.tile([128, KD, tb], BF16, tag="xt")
                nc.sync.dma_start(xt_sb[:], seg["xt"][:, :, tok0:tok0 + tb])

                h_tiles = []
                for dfc in range(NF // 4):
                    if first_seg and b == 0 and dfc == 0:
                        wg_pieces = [
                            wgpool.tile([128, KD, 128], BF16, bufs=1,
                                        tag=f"wg0p{i}", name=f"wg0p{i}")
                            for i in range(4)
                        ]
                        for i in range(4):
                            nc.sync.dma_start(
                                wg_pieces[i][:],
                                seg["wgt"][:, :, i * 128:(i + 1) * 128],
                            )
                        wg_ch = None
                    else:
                        wg_pieces = None
                        wg_ch = wgpool.tile([128, KD, 512], BF16, tag="wg")
                        nc.sync.dma_start(
                            wg_ch[:], seg["wgt"][:, :, dfc * 512:(dfc + 1) * 512]
                        )
                    if b == 0 and repeat == 1:
                        nc.sync.dma_start(
                            seg["w1_parts"][dfc][:],
                            seg["w1t"][:, :, dfc * 512:(dfc + 1) * 512],
                        )
                    for j in range(4):
                        psg = psum1.tile([128, tb], F32, tag="psg", bufs=2)
                        for d in range(KD):
                            if wg_pieces is not None:
                                wslice = wg_pieces[j][:, d, :]
                            else:
                                wslice = wg_ch[:, d, j * 128:(j + 1) * 128]
                            nc.tensor.matmul(
                                psg[:], wslice, xt_sb[:, d, :],
                                start=(d == 0), stop=(d == KD - 1),
                            )
                        ps1t = psum1.tile([128, tb], F32, tag="ps1t", bufs=2)
                        for d in range(KD):
                            nc.tensor.matmul(
                                ps1t[:],
                                seg["w1_parts"][dfc][:, d, j * 128:(j + 1) * 128],
                                xt_sb[:, d, :],
                                start=(d == 0), stop=(d == KD - 1),
                            )
                        hg = hgpool.tile([128, tb], BF16, tag="hg")
                        nc.scalar.activation(hg[:], psg[:], silu)
                        h = hpool.tile([128, tb], BF16, tag="h")
                        nc.vector.tensor_mul(h[:], hg[:], ps1t[:])
                        h_tiles.append(h)

                n_m = tb // 128
                for half in range(2):
                    psys = [
                        psum2.tile([128, 512], F32, tag="psy", name=f"psy{m}")
                        for m in range(n_m)
                    ]
                    for df in range(NF):
                        w2_ch = w2pool.tile([128, 512], BF16, tag="w2c")
                        nc.sync.dma_start(
                            w2_ch[:],
                            seg["w2t"][:, df, half * 512:(half + 1) * 512],
                        )
                        for m in range(n_m):
                            nc.tensor.matmul(
                                psys[m][:],
                                h_tiles[df][:, m * 128:(m + 1) * 128],
                                w2_ch[:],
                                start=(df == 0),
                                stop=(df == NF - 1),
                            )
                    for m in range(n_m):
                        y_sb = ypool.tile([128, 512], F32, tag="ysb")
                        nc.vector.tensor_copy(y_sb[:], psys[m][:])
                        nc.sync.dma_start(
                            seg["y"][
                                tok0 + m * 128: tok0 + (m + 1) * 128,
                                half * 512:(half + 1) * 512,
                            ],
                            y_sb[:],
                        )
                tok0 += tb

            def _trace_body():
                _seg_body(segs[0], True)
                _seg_body(segs[1], False)

            if repeat == 1:
                _trace_body()
            else:
                with tc.For_i(0, repeat, 1):
                    _trace_body()
    _split_multi_waits(nc)
    return nc


def _swizzle_k(a: np.ndarray) -> np.ndarray:
    k, f = a.shape
    return np.ascontiguousarray(a.reshape(k // 128, 128, f).transpose(1, 0, 2))


def _route(xf: np.ndarray, gate_w: np.ndarray):
    logits = xf @ gate_w.T.astype(np.float32)
    top_idx = np.argsort(-logits, axis=1, kind="stable")[:, :TOP_K]
    top_vals = np.take_along_axis(logits, top_idx, axis=1)
    m = top_vals.max(axis=1, keepdims=True)
    ex = np.exp(top_vals - m)
    top_w = ex / ex.sum(axis=1, keepdims=True)
    pair_expert = top_idx.reshape(-1)
    pair_w = top_w.reshape(-1).astype(np.float32)
    order = np.argsort(pair_expert, kind="stable")
    counts = np.bincount(pair_expert, minlength=N_EXPERTS)
    starts = np.concatenate([[0], np.cumsum(counts)])
    return pair_w, order, counts, starts


def _plan_shards(counts):
    """Deal the 16 (expert, dff-half) shards into slot A (large) / slot B."""
    padded = [max(128, -(-int(c) // 128) * 128) for c in counts]
    shards = [(e, h) for e in range(N_EXPERTS) for h in range(2)]
    shards.sort(key=lambda s: -padded[s[0]])
    slot_a, slot_b = shards[:N_CORES], shards[N_CORES:]
    CA = max(padded[e] for e, _ in slot_a)
    CB = max(padded[e] for e, _ in slot_b)
    return slot_a, slot_b, CA, CB


def _prepare_in_maps(xf, gate_w, w1, w_gate, w2, order, counts, starts):
    slot_a, slot_b, CA, CB = _plan_shards(counts)
    sels = []
    xts = {}
    for e in range(N_EXPERTS):
        sel = order[starts[e]:starts[e + 1]]
        sels.append(sel)
        xts[e] = xf[sel // TOP_K].T  # [D, n_e]
    in_maps = []
    for c in range(N_CORES):
        m = {}
        for s, (slot, C) in (("a", (slot_a, CA)), ("b", (slot_b, CB))):
            e, h = slot[c]
            lo, hi = h * DFF, (h + 1) * DFF
            xt_full = np.zeros((D, C), dtype=np.float32)
            xt_full[:, : counts[e]] = xts[e]
            m[f"xt_{s}"] = _swizzle_k(xt_full).astype(NP_BF16)
            m[f"w1t_{s}"] = _swizzle_k(
                np.ascontiguousarray(w1[e][lo:hi].T).astype(np.float32)
            ).astype(NP_BF16)
            m[f"wgt_{s}"] = _swizzle_k(
                np.ascontiguousarray(w_gate[e][lo:hi].T).astype(np.float32)
            ).astype(NP_BF16)
            m[f"w2t_{s}"] = _swizzle_k(
                np.ascontiguousarray(w2[e][:, lo:hi].T).astype(np.float32)
            ).astype(NP_BF16)
        in_maps.append(m)
    return in_maps, (slot_a, slot_b, CA, CB, sels)


def kernel(x, gate_w, w1, w_gate, w2):
    b, t, d = x.shape
    xf = np.ascontiguousarray(x.reshape(-1, d)).astype(np.float32)
    n_tok = xf.shape[0]

    pair_w, order, counts, starts = _route(xf, gate_w)
    in_maps, (slot_a, slot_b, CA, CB, sels) = _prepare_in_maps(
        xf, gate_w, w1, w_gate, w2, order, counts, starts
    )

    key = (CA, CB)
    if key not in _NC_CACHE:
        _NC_CACHE[key] = _build_kernel(CA, CB)
    nc = _NC_CACHE[key]

    res = run_bass_kernel_spmd(nc, in_maps, core_ids=list(range(N_CORES)))

    # Sum the two dff-half partials per expert, then router-weighted combine.
    y_full = {e: np.zeros((counts[e], D), dtype=np.float32) for e in range(N_EXPERTS)}
    for c in range(N_CORES):
        for s, slot in (("a", slot_a), ("b", slot_b)):
            e, _h = slot[c]
            y_full[e] += res.results[c][f"y_{s}"][: counts[e]]

    contrib = np.zeros((n_tok * TOP_K, D), dtype=np.float32)
    for e in range(N_EXPERTS):
        sel = sels[e]
        contrib[sel] = y_full[e] * pair_w[sel][:, None]
    out = contrib.reshape(n_tok, TOP_K, D).sum(axis=1)
    return out.reshape(b, t, d).astype(x.dtype)


# revision 8
# speedup vs baseline: 1.6860x; 1.6860x over previous
"""MoE layer (top-2 of 8, SwiGLU) on 8 TRN2 cores — dff-split load balancing.

Each expert's FFN is split into two d_ff halves (2048 each); the 16
(expert, half) shards are sorted by padded token count and dealt so every
core gets one large-slot shard (C_A tokens) and one small-slot shard
(C_B tokens).  Pairing big experts' shards with small ones cuts the
critical core from 2176 full-token equivalents to (C_A + C_B)/2 = 2112.
SwiGLU is elementwise in d_ff, so each shard's partial y is exact; the
host sums the two halves per expert before the router-weighted combine.
"""

import numpy as np
import ml_dtypes

import concourse.bass as bass
import concourse.mybir as mybir
import concourse.tile as tile
from concourse.bass_utils import run_bass_kernel_spmd

_ws_counter = [0]


def _split_multi_waits(nc: bass.Bass) -> int:
    n_split = 0
    for f in nc.m.functions:
        for bb in f.blocks:
            new_insts = []
            for inst in bb.instructions:
                si = inst.sync_info
                if si is not None and si.on_wait and len(si.on_wait) > 1:
                    waits = list(si.on_wait)
                    for w in waits[:-1]:
                        _ws_counter[0] += 1
                        n_split += 1
                        new_insts.append(
                            mybir.InstNoOp(
                                name=f"waitsplit-{_ws_counter[0]}",
                                opcode="NoOp",
                                engine=inst.engine,
                                sync_info=mybir.SyncInfo(on_wait=[w], on_update=[]),
                                bass_nofuse=True,
                                text_hint="waitsplit",
                            )
                        )
                    si.on_wait = [waits[-1]]
                new_insts.append(inst)
            bb.instructions[:] = new_insts
    return n_split


D = 1024
DFF = 2048          # per-shard d_ff half
N_EXPERTS = 8
TOP_K = 2
N_CORES = 8
TB = 512
KD = D // 128       # 8 contraction tiles over d
NF = DFF // 128     # 16 tiles over the d_ff half

BF16 = mybir.dt.bfloat16
F32 = mybir.dt.float32
NP_BF16 = ml_dtypes.bfloat16

_NC_CACHE: dict[tuple, bass.Bass] = {}


def _blocks_for(C: int) -> list[int]:
    blocks = [TB] * (C // TB)
    r = C % TB
    if r == 128:
        if blocks:
            blocks = blocks[:-1] + [384, 256]
        else:
            blocks = [128]
    elif r:
        blocks.append(r)
    blocks.sort()
    return blocks


def _build_kernel(CA: int, CB: int, repeat: int = 1) -> bass.Bass:
    assert CA % 128 == 0 and CB % 128 == 0
    nc = bass.Bass()
    segs = []
    for s, C in (("a", CA), ("b", CB)):
        segs.append(
            dict(
                C=C,
                xt=nc.dram_tensor(f"xt_{s}", [128, KD, C], BF16, kind="ExternalInput"),
                w1t=nc.dram_tensor(f"w1t_{s}", [128, KD, DFF], BF16, kind="ExternalInput"),
                wgt=nc.dram_tensor(f"wgt_{s}", [128, KD, DFF], BF16, kind="ExternalInput"),
                w2t=nc.dram_tensor(f"w2t_{s}", [128, NF, D], BF16, kind="ExternalInput"),
                y=nc.dram_tensor(f"y_{s}", [C, D], F32, kind="ExternalOutput"),
            )
        )

    silu = mybir.ActivationFunctionType.Silu

    with tile.TileContext(nc) as tc:
        with (
            tc.tile_pool(name="wres", bufs=1) as wres,
            tc.tile_pool(name="wg", bufs=4) as wgpool,
            tc.tile_pool(name="xt", bufs=3) as xtpool,
            tc.tile_pool(name="hg", bufs=3) as hgpool,
            tc.tile_pool(name="h", bufs=24) as hpool,
            tc.tile_pool(name="w2", bufs=12) as w2pool,
            tc.tile_pool(name="yo", bufs=4) as ypool,
            tc.tile_pool(name="ps1", bufs=1, space="PSUM") as psum1,
            tc.tile_pool(name="ps2", bufs=4, space="PSUM") as psum2,
        ):
            # Resident w1 per segment, in 512-col parts (NF//4 = 4 per seg).
            for si, seg in enumerate(segs):
                seg["w1_parts"] = [
                    wres.tile([128, KD, 512], BF16, tag=f"w1p{si}_{i}",
                              name=f"w1p{si}_{i}")
                    for i in range(NF // 4)
                ]

            if repeat > 1:
                for seg in segs:
                    for i in range(NF // 4):
                        nc.sync.dma_start(
                            seg["w1_parts"][i][:],
                            seg["w1t"][:, :, i * 512:(i + 1) * 512],
                        )

            def _seg_body(seg, first_seg):
              tok0 = 0
              for b, tb in enumerate(_blocks_for(seg["C"])):
                xt_sb = xtpool.tile([128, KD, tb], BF16, tag="xt")
                nc.sync.dma_start(xt_sb[:], seg["xt"][:, :, tok0:tok0 + tb])

                h_tiles = []
                for dfc in range(NF // 4):
                    if first_seg and b == 0 and dfc == 0:
                        wg_pieces = [
                            wgpool.tile([128, KD, 128], BF16, bufs=1,
                                        tag=f"wg0p{i}", name=f"wg0p{i}")
                            for i in range(4)
                        ]
                        for i in range(4):
                            nc.sync.dma_start(
                                wg_pieces[i][:],
                                seg["wgt"][:, :, i * 128:(i + 1) * 128],
                            )
                        wg_ch = None
                    else:
                        wg_pieces = None
                        wg_ch = wgpool.tile([128, KD, 512], BF16, tag="wg")
                        nc.sync.dma_start(
                            wg_ch[:], seg["wgt"][:, :, dfc * 512:(dfc + 1) * 512]
                        )
                    if b == 0 and repeat == 1:
                        nc.sync.dma_start(
                            seg["w1_parts"][dfc][:],
                            seg["w1t"][:, :, dfc * 512:(dfc + 1) * 512],
                        )
                    for j in range(4):
                        psg = psum1.tile([128, tb], F32, tag="psg", bufs=2)
                        for d in range(KD):
                            if wg_pieces is not None:
                                wslice = wg_pieces[j][:, d, :]
                            else:
                                wslice = wg_ch[:, d, j * 128:(j + 1) * 128]
                            nc.tensor.matmul(
                                psg[:], wslice, xt_sb[:, d, :],
                                start=(d == 0), stop=(d == KD - 1),
                            )
                        ps1t = psum1.tile([128, tb], F32, tag="ps1t", bufs=2)
                        for d in range(KD):
                            nc.tensor.matmul(
                                ps1t[:],
                                seg["w1_parts"][dfc][:, d, j * 128:(j + 1) * 128],
                                xt_sb[:, d, :],
                                start=(d == 0), stop=(d == KD - 1),
                            )
                        hg = hgpool.tile([128, tb], BF16, tag="hg")
                        nc.scalar.activation(hg[:], psg[:], silu)
                        h = hpool.tile([128, tb], BF16, tag="h")
                        nc.vector.tensor_mul(h[:], hg[:], ps1t[:])
                        h_tiles.append(h)

                n_m = tb // 128
                for half in range(2):
                    psys = [
                        psum2.tile([128, 512], F32, tag="psy", name=f"psy{m}")
                        for m in range(n_m)
                    ]
                    for df in range(NF):
                        w2_ch = w2pool.tile([128, 512], BF16, tag="w2c")
                        nc.sync.dma_start(
                            w2_ch[:],
                            seg["w2t"][:, df, half * 512:(half + 1) * 512],
                        )
                        for m in range(n_m):
                            nc.tensor.matmul(
                                psys[m][:],
                                h_tiles[df][:, m * 128:(m + 1) * 128],
                                w2_ch[:],
                                start=(df == 0),
                                stop=(df == NF - 1),
                            )
                    for m in range(n_m):
                        y_sb = ypool.tile([128, 512], F32, tag="ysb")
                        nc.vector.tensor_copy(y_sb[:], psys[m][:])
                        nc.sync.dma_start(
                            seg["y"][
                                tok0 + m * 128: tok0 + (m + 1) * 128,
                                half * 512:(half + 1) * 512,
                            ],
                            y_sb[:],
                        )
                tok0 += tb

            def _trace_body():
                _seg_body(segs[0], True)
                _seg_body(segs[1], False)

            if repeat == 1:
                _trace_body()
            else:
                with tc.For_i(0, repeat, 1):
                    _trace_body()
    _split_multi_waits(nc)
    return nc


def _swizzle_k(a: np.ndarray) -> np.ndarray:
    k, f = a.shape
    return np.ascontiguousarray(a.reshape(k // 128, 128, f).transpose(1, 0, 2))


def _route(xf: np.ndarray, gate_w: np.ndarray):
    logits = xf @ gate_w.T.astype(np.float32)
    top_idx = np.argsort(-logits, axis=1, kind="stable")[:, :TOP_K]
    top_vals = np.take_along_axis(logits, top_idx, axis=1)
    m = top_vals.max(axis=1, keepdims=True)
    ex = np.exp(top_vals - m)
    top_w = ex / ex.sum(axis=1, keepdims=True)
    pair_expert = top_idx.reshape(-1)
    pair_w = top_w.reshape(-1).astype(np.float32)
    order = np.argsort(pair_expert, kind="stable")
    counts = np.bincount(pair_expert, minlength=N_EXPERTS)
    starts = np.concatenate([[0], np.cumsum(counts)])
    return pair_w, order, counts, starts


def _plan_shards(counts):
    """Deal the 16 (expert, dff-half) shards into slot A (large) / slot B."""
    padded = [max(128, -(-int(c) // 128) * 128) for c in counts]
    shards = [(e, h) for e in range(N_EXPERTS) for h in range(2)]
    shards.sort(key=lambda s: -padded[s[0]])
    slot_a, slot_b = shards[:N_CORES], shards[N_CORES:]
    CA = max(padded[e] for e, _ in slot_a)
    CB = max(padded[e] for e, _ in slot_b)
    return slot_a, slot_b, CA, CB


def _prepare_in_maps(xf, gate_w, w1, w_gate, w2, order, counts, starts):
    slot_a, slot_b, CA, CB = _plan_shards(counts)
    sels = []
    xts = {}
    for e in range(N_EXPERTS):
        sel = order[starts[e]:starts[e + 1]]
        sels.append(sel)
        xts[e] = xf[sel // TOP_K].T  # [D, n_e]
    in_maps = []
    for c in range(N_CORES):
        m = {}
        for s, (slot, C) in (("a", (slot_a, CA)), ("b", (slot_b, CB))):
            e, h = slot[c]
            lo, hi = h * DFF, (h + 1) * DFF
            xt_full = np.zeros((D, C), dtype=np.float32)
            xt_full[:, : counts[e]] = xts[e]
            m[f"xt_{s}"] = _swizzle_k(xt_full).astype(NP_BF16)
            m[f"w1t_{s}"] = _swizzle_k(
                np.ascontiguousarray(w1[e][lo:hi].T).astype(np.float32)
            ).astype(NP_BF16)
            m[f"wgt_{s}"] = _swizzle_k(
                np.ascontiguousarray(w_gate[e][lo:hi].T).astype(np.float32)
            ).astype(NP_BF16)
            m[f"w2t_{s}"] = _swizzle_k(
                np.ascontiguousarray(w2[e][:, lo:hi].T).astype(np.float32)
            ).astype(NP_BF16)
        in_maps.append(m)
    return in_maps, (slot_a, slot_b, CA, CB, sels)


def kernel(x, gate_w, w1, w_gate, w2):
    b, t, d = x.shape
    xf = np.ascontiguousarray(x.reshape(-1, d)).astype(np.float32)
    n_tok = xf.shape[0]

    pair_w, order, counts, starts = _route(xf, gate_w)
    in_maps, (slot_a, slot_b, CA, CB, sels) = _prepare_in_maps(
        xf, gate_w, w1, w_gate, w2, order, counts, starts
    )

    key = (CA, CB)
    if key not in _NC_CACHE:
        _NC_CACHE[key] = _build_kernel(CA, CB)
    nc = _NC_CACHE[key]

    res = run_bass_kernel_spmd(nc, in_maps, core_ids=list(range(N_CORES)))

    # Sum the two dff-half partials per expert, then router-weighted combine.
    y_full = {e: np.zeros((counts[e], D), dtype=np.float32) for e in range(N_EXPERTS)}
    for c in range(N_CORES):
        for s, slot in (("a", slot_a), ("b", slot_b)):
            e, _h = slot[c]
            y_full[e] += res.results[c][f"y_{s}"][: counts[e]]

    contrib = np.zeros((n_tok * TOP_K, D), dtype=np.float32)
    for e in range(N_EXPERTS):
        sel = sels[e]
        contrib[sel] = y_full[e] * pair_w[sel][:, None]
    out = contrib.reshape(n_tok, TOP_K, D).sum(axis=1)
    return out.reshape(b, t, d).astype(x.dtype)
